# revision 8
# baseline (speedup 1.0000x reference)
"""Euclidean distance block (retrieval kNN) on 8 TRN2 NeuronCores.

dist[b, s, p] = sqrt(sum_c (x1[b, c, p] - x2[b, s, c, p])^2)   p = spatial (h*w)
out[b] = dist[b].reshape(S * h * w)

Sharding: data-parallel over batch B=32 -> 4 batches per core, no comms.

Per-core layout (spatial-split): SBUF partitions carry (channel, spatial_half)
= 64*2 = 128; the free axis carries the 882 spatial positions of one half.
Every tensor reshapes cleanly onto 128 partitions:
  x1[b]  -> [128, 882]      one DMA per batch, no partition-duplicate copy
  x2[b,s]-> [128, 882]      streamed as [128, 2, 882] two-support tiles
  out[b] -> rows (s, si)    [50, 882] f32, contiguous 3528 B per row

Compute chain per support: DVE subtract in bf16 (2x mode, in place), Square
split between ACT and DVE (cadence balance), then one [128, 50] one-hot mask
matmul per spatial half accumulating sum-over-C into PSUM [50, 441] (rows =
(s, si)); supports accumulate start/stop over s = 0..24.  LDWEIGHTS streams on
PE's second read port so per-support mask reloads pipeline behind the matmuls.
ACT Sqrt PSUM -> SBUF f32, one 176 KB store per batch on the otherwise-idle
Sync HWDGE ring (loads-with-cast must use the GpSimd SWDGE ring, and a store
queued behind loads on the same ring would stall them).
"""

import numpy as np

B, S, C, H, W = 32, 25, 64, 42, 42
HW = H * W            # 1764
PHALF = HW // 2       # 882
QW = PHALF // 2       # 441
NCORES = 8
BL = B // NCORES      # 4 batches per core
NSO = 2               # supports per streamed tile
NTILE = S // NSO      # 12 two-support tiles, then 1 leftover single

_cache = {}


def _build_nc():
    import concourse.bacc as bacc
    import concourse.mybir as mybir
    from concourse.tile import TileContext
    from concourse.bass import MemorySpace

    f32 = mybir.dt.float32
    bf16 = mybir.dt.bfloat16
    Square = mybir.ActivationFunctionType.Square
    Sqrt = mybir.ActivationFunctionType.Sqrt
    sub = mybir.AluOpType.subtract
    mult = mybir.AluOpType.mult

    # Square and Sqrt both live in the "sqrt_and_others" act-function set,
    # but the table-load chooser picks the first set containing each one,
    # alternating two ~2.7us table reloads per batch. Strip the two
    # functions from every other set (contents only — set ids are
    # positional) so one resident table serves the whole kernel.
    _orig_tables = bacc.get_activation_tables

    def _pinned_tables(arch):
        t = _orig_tables(arch)
        for name, fns in t.items():
            if name != "sqrt_and_others":
                fns.discard(Square)
                fns.discard(Sqrt)
        return t

    bacc.get_activation_tables = _pinned_tables
    nc = bacc.Bacc()
    x1 = nc.declare_dram_parameter("x1", [BL, C, HW], f32, isOutput=False)
    x2 = nc.declare_dram_parameter("x2", [BL, S, C, HW], f32, isOutput=False)
    mk = nc.declare_dram_parameter("mask", [128, S, 2 * S], bf16, isOutput=False)
    out = nc.declare_dram_parameter("out", [BL, S * HW], f32, isOutput=True)

    with TileContext(nc) as tc:
        with (
            tc.tile_pool(name="x2p", bufs=16) as x2p,
            tc.tile_pool(name="x2lp", bufs=2) as x2lp,
            tc.tile_pool(name="sqp", bufs=8) as sqp,
            tc.tile_pool(name="x1p", bufs=2) as x1p,
            tc.tile_pool(name="outp", bufs=2) as outp,
            tc.tile_pool(name="cst", bufs=1) as cst,
            tc.tile_pool(name="ps", bufs=3, space=MemorySpace.PSUM) as psp,
        ):
            # mask rows are (s, si): mt[k, s, 2s+si(k)] = 1; contiguous
            # 2500 B per partition, one clean HWDGE load
            mt = cst.tile([128, S, 2 * S], bf16)
            nc.sync.dma_start(mt[:], mk[:, :, :])

            prev = None
            for b in range(BL):
                # x1[b]: partition (c, si), one contiguous 3528 B run per
                # partition, cast-loaded just ahead of its batch's stream
                x1bt = x1p.tile([128, PHALF], bf16, tag="x1b")
                nc.gpsimd.dma_start(
                    x1bt[:], x1[b].rearrange("c (si p) -> (c si) p", si=2)
                )
                # stream all supports of the batch: 12 two-support tiles
                # plus the odd support 24 as a single-support tile
                tiles = []
                for i in range(NTILE):
                    x2t = x2p.tile([128, NSO, PHALF], bf16, tag="x2t")
                    nc.gpsimd.dma_start(
                        x2t[:],
                        x2[b, NSO * i : NSO * (i + 1)].rearrange(
                            "s c (si p) -> (c si) s p", si=2
                        ),
                    )
                    tiles.append(x2t)
                x2l = x2lp.tile([128, 1, PHALF], bf16, tag="x2l")
                nc.gpsimd.dma_start(
                    x2l[:], x2[b, S - 1 :].rearrange("s c (si p) -> (c si) s p", si=2)
                )

                x1b = x1bt[:, :]
                pst = [
                    psp.tile([2 * S, QW], f32, name=f"ps{h}", tag=f"ps{h}")
                    for h in range(2)
                ]

                def flush_prev(prev=prev):
                    # emit the previous batch's sqrt + store mid-way into
                    # this batch's loop: by then its last matmul has long
                    # retired, so ACT slots the sqrts between squares
                    # instead of head-of-line blocking on the accumulation
                    if prev is None:
                        return
                    ppst, pb = prev
                    ot = outp.tile([2 * S, PHALF], f32, name="ot", tag="ot")
                    for h in range(2):
                        nc.scalar.activation(
                            ot[:, h * QW : (h + 1) * QW], ppst[h][:], Sqrt
                        )
                    nc.sync.dma_start(
                        out[pb].rearrange("(s si p) -> (s si) p", si=2, p=PHALF),
                        ot[:],
                    )

                for s in range(S):
                    if s == 4:
                        flush_prev()
                    if s < S - 1:
                        d = tiles[s // NSO][:, s % NSO, :]
                    else:
                        d = x2l[:, 0, :]
                    nc.vector.tensor_tensor(d, d, x1b, sub)
                    sq = sqp.tile([128, PHALF], bf16, tag="sq")
                    # squares split 2/1 ACT/DVE to balance engine cadence
                    if s % 3 != 2:
                        nc.scalar.activation(sq[:], d, Square)
                    else:
                        nc.vector.tensor_tensor(sq[:], d, d, mult)
                    for h in range(2):
                        nc.tensor.matmul(
                            pst[h][:, :],
                            mt[:, s, :],
                            sq[:, h * QW : (h + 1) * QW],
                            start=(s == 0),
                            stop=(s == S - 1),
                        )
                prev = (pst, b)

            # final batch: sqrt + store immediately, split per half so the
            # h0 store overlaps the h1 sqrt
            ppst, pb = prev
            ot = outp.tile([2 * S, PHALF], f32, name="ot", tag="ot")
            dst = out[pb].rearrange("(s si p) -> (s si) p", si=2, p=PHALF)
            for h in range(2):
                nc.scalar.activation(ot[:, h * QW : (h + 1) * QW], ppst[h][:], Sqrt)
                nc.sync.dma_start(
                    dst[:, h * QW : (h + 1) * QW], ot[:, h * QW : (h + 1) * QW]
                )

    try:
        nc.finalize()
    finally:
        bacc.get_activation_tables = _orig_tables
    return nc


def get_nc():
    if "nc" not in _cache:
        _cache["nc"] = _build_nc()
    return _cache["nc"]


def make_mask() -> np.ndarray:
    # mask[k, s, m] = 1 iff the sum of partition k (= channel c = k//2,
    # spatial half si = k%2) for support s belongs to output row m = 2s + si.
    import ml_dtypes

    mask = np.zeros((128, S, 2 * S), dtype=ml_dtypes.bfloat16)
    for k in range(128):
        si = k % 2
        for s in range(S):
            mask[k, s, 2 * s + si] = 1.0
    return mask


def make_in_maps(x1: np.ndarray, x2: np.ndarray) -> list[dict]:
    x1 = np.ascontiguousarray(np.asarray(x1, dtype=np.float32)).reshape(B, C, HW)
    x2 = np.ascontiguousarray(np.asarray(x2, dtype=np.float32)).reshape(B, S, C, HW)
    mask = make_mask()
    maps = []
    for i in range(NCORES):
        sl = slice(i * BL, (i + 1) * BL)
        maps.append({"x1": x1[sl], "x2": x2[sl], "mask": mask})
    return maps


def gather_out(results: list[dict]) -> np.ndarray:
    return np.concatenate([np.asarray(r["out"]) for r in results], axis=0).astype(
        np.float32, copy=False
    )


def kernel(x1, x2) -> np.ndarray:
    from concourse.bass_utils import run_bass_kernel_spmd

    nc = get_nc()
    in_maps = make_in_maps(x1, x2)
    res = run_bass_kernel_spmd(nc, in_maps, list(range(NCORES)))
    return gather_out(res.results)
